# revision 8
# baseline (speedup 1.0000x reference)
"""Trainium2 Bass kernel for a 4-layer CUBA-LIF spiking network.

Per layer: z[t] = W @ s[t]; cur = 0.75*cur + z[t]; v = 0.97*v + cur;
s = (v >= 1.25); v *= (1-s).  Returns (spikes4 [64,35,1000], counts [1,4]).

Sharding: data-parallel over batch, 8 batches per NeuronCore x 8 cores.

Per-core schedule: time is processed in Tc=25-step chunks with the four
layers pipelined at a 2-chunk skew.  One fused custom-DVE instruction per
timestep advances the packed membrane state of all four layers at once:

    v_pre[t] = (v_pre[t-1] * (v_pre[t-1] < theta)) * beta + cur[t]

PE runs the synapse matmuls (fp32), ScalarE copies PSUM->SBUF, Pool runs
segmented current-integration scans (cur = 0.75*cur + z along time, spacer
columns carry inter-chunk state) plus spike thresholding with fused count
reduction, VectorE runs only the serial LIF wave.  Spikes are recovered in
bulk from the stored v_pre trajectory.

State packing: free index f = (l-1)*16 + h*8 + b for layer l, output-half
h (neuron o = h*128 + partition), batch b.  Layer 4 uses h=0, partitions
0..34 only; its dead lanes compute on zeroed/stale-but-finite data and are
never read.
"""

import numpy as np

import concourse.bacc as bacc
import concourse.mybir as mybir
import concourse.dve_ops as dve_ops
from concourse.dve_ops import DveOp
from concourse.dve_spec import Spec, Src0, Src1, C0, C1, lower, _has_src1
from concourse.dve_uop import DveOpSpec
from concourse.tile import TileContext
from concourse.bass_utils import run_bass_kernel_spmd

N_CORES = 8
B_TOT, T = 64, 1000
B = B_TOT // N_CORES              # 8 batches per core
DIMS = [20, 256, 256, 256, 35]
NL = 4
TC = 25                           # timesteps per chunk
NCH = T // TC                     # 40 chunks
SKEW = 2                          # chunk skew between consecutive layers
NW = NCH + SKEW * (NL - 1)        # 46 waves
F = NL * 16                       # packed (layer, half, batch) free width
ALPHA = 0.75
BETA = float(np.float32(1.0 - 0.03))
THETA = 1.25
DT = mybir.dt.float32
AL = mybir.AluOpType


def _lif_ref(in0, in1, s0, s1, imm2):
    return (in0 * (in0 < s0)) * s1 + in1


def _register_lif_op() -> DveOp:
    name = "LIF_STEP_ANT"
    for op in dve_ops.OPS:
        if op.name == name:
            return op
    spec = Spec(body=(Src0 * (Src0 < C0)) * C1 + Src1, reference=_lif_ref)
    row = max(dve_ops._SUB_OPCODE_FOR_NAME.values()) + 1
    assert row < 0x20
    shas = {}
    for ver in ("v3", "v4"):
        s = DveOpSpec(name=name, opcode=row, uops=lower(spec, ver=ver),
                      rd1_en=_has_src1(spec))
        shas[ver] = s.sha(ver)
    op = DveOp(name, spec, subdim=False, uops_sha=shas)
    dve_ops._SUB_OPCODE_FOR_NAME[name] = row
    dve_ops.OPS.append(op)
    dve_ops.CUSTOM_DVE_SPECS[name] = spec
    return op


def _valid_layers(w):
    return [l for l in range(1, NL + 1) if 0 <= w - SKEW * (l - 1) < NCH]


def _flat(ap):
    return ap.rearrange("p a b -> p (a b)")


def _build():
    lif = _register_lif_op()
    nc = bacc.Bacc("TRN2", target_bir_lowering=False, debug=False)

    sp_in = nc.declare_dram_parameter("sp_in", [DIMS[0], NCH, B * TC], DT,
                                      isOutput=False)
    wt_d = [nc.declare_dram_parameter(f"w{l}t", [DIMS[l - 1], DIMS[l]], DT,
                                      isOutput=False)
            for l in range(1, NL + 1)]
    s4_out = nc.declare_dram_parameter("s4_out", [35, NCH, B * TC], DT,
                                       isOutput=True)
    cnt_out = nc.declare_dram_parameter("cnt_out", [128, NL - 1, 16 * TC], DT,
                                        isOutput=True)

    with TileContext(nc) as tc:
        with (
            tc.tile_pool(name="wts", bufs=1) as wts,
            tc.tile_pool(name="state", bufs=1) as state,
            tc.tile_pool(name="psum", bufs=2, space="PSUM") as psum,
        ):
            # persistent weights: w1t [20,256]; w2t/w3t [128,2(Kh),256];
            # w4t [128,2(Kh),35]
            w1t = wts.tile([DIMS[0], 256], DT, name="w1t_s")
            w2t = wts.tile([128, 2, 256], DT, name="w2t_s")
            w3t = wts.tile([128, 2, 256], DT, name="w3t_s")
            w4t = wts.tile([128, 2, 35], DT, name="w4t_s")
            alpha = state.tile([128, F, TC + 1], DT, name="alpha_pat")
            zbuf = [state.tile([128, F, TC + 1], DT, name=f"zbuf{i}")
                    for i in range(3)]
            curb = [state.tile([128, F, TC + 1], DT, name=f"curb{i}")
                    for i in range(3)]
            vpre = [state.tile([128, F, TC], DT, name=f"vpre{i}")
                    for i in range(2)]
            sbft = [state.tile([128, F, TC], DT, name=f"sbft{i}")
                    for i in range(2)]
            spin = [state.tile([DIMS[0], B * TC], DT, name=f"spin{i}")
                    for i in range(3)]
            cnt = state.tile([128, NL - 1, 16 * TC], DT, name="cnt")

            nc.sync.dma_start(out=w1t[:], in_=wt_d[0][:])
            for wtile, d in ((w2t, wt_d[1]), (w3t, wt_d[2]), (w4t, wt_d[3])):
                nc.sync.dma_start(out=wtile[:, 0, :], in_=d[0:128, :])
                nc.sync.dma_start(out=wtile[:, 1, :], in_=d[128:256, :])

            nc.gpsimd.memset(alpha[:], ALPHA)
            nc.gpsimd.memset(alpha[:, :, 0], 0.0)
            for t_ in curb:
                nc.gpsimd.memset(t_[:], 0.0)
            for t_ in zbuf:
                nc.gpsimd.memset(t_[:], 0.0)
            for t_ in vpre:
                nc.vector.memset(t_[:], 0.0)
            nc.vector.memset(cnt[:], 0.0)

            for c in range(min(3, NCH)):
                nc.sync.dma_start(out=spin[c % 3][:], in_=sp_in[:, c, :])

            def emit_post(w):
                """Threshold wave-w vpre into spikes + counts; DMA layer-4."""
                for l in _valid_layers(w):
                    c = w - SKEW * (l - 1)
                    f0 = (l - 1) * 16
                    if l < NL:
                        nc.gpsimd.tensor_scalar(
                            _flat(sbft[w % 2][:, f0:f0 + 16, :]),
                            _flat(vpre[w % 2][:, f0:f0 + 16, :]),
                            THETA, None, AL.is_ge)
                        nc.gpsimd.tensor_tensor(
                            cnt[:, l - 1, :], cnt[:, l - 1, :],
                            _flat(sbft[w % 2][:, f0:f0 + 16, :]), AL.add)
                    else:
                        nc.gpsimd.tensor_scalar(
                            _flat(sbft[w % 2][0:35, f0:f0 + 8, :]),
                            _flat(vpre[w % 2][0:35, f0:f0 + 8, :]),
                            THETA, None, AL.is_ge)
                        nc.sync.dma_start(
                            out=s4_out[:, c, :],
                            in_=_flat(sbft[w % 2][0:35, f0:f0 + 8, :]))

            def emit_prep(w):
                """Matmuls + PSUM copies + carries + scans feeding wave w."""
                for l in _valid_layers(w):
                    c = w - SKEW * (l - 1)
                    f0 = (l - 1) * 16
                    # synapse matmul -> psum
                    if l == 1:
                        zp = psum.tile([128, 2, B * TC], DT, name=f"zp1_{w}",
                                       tag="zp1")
                        for oh in range(2):
                            nc.tensor.matmul(zp[:, oh, :],
                                             w1t[:, oh * 128:(oh + 1) * 128],
                                             spin[c % 3][:],
                                             start=True, stop=True)
                    elif l < NL:
                        zp = psum.tile([128, 2, B * TC], DT, name=f"zp{l}_{w}",
                                       tag=f"zp{l}")
                        wtile = w2t if l == 2 else w3t
                        sp = sbft[(w - SKEW) % 2]
                        fp = (l - 2) * 16
                        for oh in range(2):
                            for kh in range(2):
                                nc.tensor.matmul(
                                    zp[:, oh, :],
                                    wtile[:, kh, oh * 128:(oh + 1) * 128],
                                    _flat(sp[:, fp + kh * 8:fp + kh * 8 + 8, :]),
                                    start=(kh == 0), stop=(kh == 1))
                    else:
                        zp = psum.tile([35, B * TC], DT, name=f"zp4_{w}",
                                       tag="zp4")
                        sp = sbft[(w - SKEW) % 2]
                        fp = (NL - 2) * 16
                        for kh in range(2):
                            nc.tensor.matmul(
                                zp[:],
                                w4t[:, kh, :],
                                _flat(sp[:, fp + kh * 8:fp + kh * 8 + 8, :]),
                                start=(kh == 0), stop=(kh == 1))
                    # PSUM -> zbuf cols 1..TC (ScalarE)
                    zb = zbuf[w % 3]
                    if l < NL:
                        for oh in range(2):
                            nc.scalar.copy(
                                out=zb[:, f0 + oh * 8:f0 + oh * 8 + 8, 1:],
                                in_=zp[:, oh, :].rearrange(
                                    "p (b t) -> p b t", b=B))
                    else:
                        nc.scalar.copy(
                            out=zb[0:35, f0:f0 + 8, 1:],
                            in_=zp[:].rearrange("p (b t) -> p b t", b=B))
                    # carry spacers from previous chunk's cur (ScalarE)
                    nc.scalar.copy(out=zb[:, f0:f0 + 16, 0:1],
                                   in_=curb[(w - 1) % 3][:, f0:f0 + 16,
                                                         TC:TC + 1])
                # one merged segmented current scan over the contiguous
                # valid-layer range (VectorE; Pool lacks the scan opcode)
                vls = _valid_layers(w)
                fa = (vls[0] - 1) * 16
                fb = vls[-1] * 16
                nc.vector.tensor_tensor_scan(
                    _flat(curb[w % 3][:, fa:fb, :]),
                    _flat(alpha[:, 0:fb - fa, :]),
                    _flat(zbuf[w % 3][:, fa:fb, :]),
                    0.0, AL.mult, AL.add)
                # prefetch input chunk w+3 (after the layer-1 matmul has
                # consumed this wave's slot — emission order sets the dep)
                if w + 3 < NCH:
                    nc.sync.dma_start(out=spin[(w + 3) % 3][:],
                                      in_=sp_in[:, w + 3, :])

            def emit_wave(w):
                for j in range(TC):
                    if j > 0:
                        prev = vpre[w % 2][:, :, j - 1]
                    else:
                        prev = vpre[(w - 1) % 2][:, :, TC - 1]
                    nc.vector._custom_dve(
                        lif,
                        out=vpre[w % 2][:, :, j],
                        in0=prev,
                        in1=curb[w % 3][:, :, j + 1],
                        s0=THETA, s1=BETA)

            for w in range(NW + 2):
                if w >= 2:
                    emit_post(w - 2)
                if w < NW:
                    emit_prep(w)
                    emit_wave(w)

            nc.sync.dma_start(out=cnt_out[:], in_=cnt[:])

    nc.compile()
    return nc


_NC_CACHE = None


def _get_nc():
    global _NC_CACHE
    if _NC_CACHE is None:
        _NC_CACHE = _build()
    return _NC_CACHE


def kernel(spike, W1, W2, W3, W4):
    spike = np.ascontiguousarray(np.asarray(spike, dtype=np.float32))
    wts = {}
    for l, W in enumerate((W1, W2, W3, W4), start=1):
        wts[f"w{l}t"] = np.ascontiguousarray(
            np.asarray(W, dtype=np.float32).T)

    in_maps = []
    for i in range(N_CORES):
        sp = spike[i * B:(i + 1) * B]                      # [B, 20, T]
        sp = sp.reshape(B, DIMS[0], NCH, TC)
        sp = sp.transpose(1, 2, 0, 3).reshape(DIMS[0], NCH, B * TC)
        in_maps.append({"sp_in": np.ascontiguousarray(sp), **wts})

    nc = _get_nc()
    res = run_bass_kernel_spmd(nc, in_maps, list(range(N_CORES)))

    s4 = np.empty((B_TOT, 35, T), dtype=np.float32)
    cnt_sums = np.zeros(NL, dtype=np.float64)
    for i in range(N_CORES):
        r = res.results[i]
        o = r["s4_out"].reshape(35, NCH, B, TC)            # [35, NCH, B, TC]
        s4[i * B:(i + 1) * B] = (
            o.transpose(2, 0, 1, 3).reshape(B, 35, T))
        cnt_sums[:NL - 1] += r["cnt_out"].astype(np.float64).sum(axis=(0, 2))
    cnt_sums[NL - 1] = s4.sum(dtype=np.float64)

    denom = np.array([B_TOT * DIMS[l] * T for l in range(1, NL + 1)],
                     dtype=np.float64)
    counts = (cnt_sums / denom).astype(np.float32).reshape(1, NL)
    return s4, counts


# revision 11
# speedup vs baseline: 1.1102x; 1.1102x over previous
"""Trainium2 Bass kernel for a 4-layer CUBA-LIF spiking network.

Per layer: z[t] = W @ s[t]; cur = 0.75*cur + z[t]; v = 0.97*v + cur;
s = (v >= 1.25); v *= (1-s).  Returns (spikes4 [64,35,1000], counts [1,4]).

Sharding: data-parallel over batch, 8 batches per NeuronCore x 8 cores.

Per-core schedule: time is processed in Tc=25-step chunks with the four
layers pipelined at a 2-chunk skew.  One fused custom-DVE instruction per
timestep advances the packed membrane state of all four layers at once:

    v_pre[t] = (v_pre[t-1] * (v_pre[t-1] < theta)) * beta + cur[t]

PE runs the synapse matmuls (fp32), ScalarE copies PSUM->SBUF, Pool does
spike thresholding and count accumulation, VectorE runs the serial LIF
wave plus one merged segmented current-integration scan per wave
(cur = 0.75*cur + z along time; spacer columns carry inter-chunk state,
and a zero in the multiplier pattern restarts the recurrence at each
segment boundary).  Spikes are recovered in bulk from the stored v_pre
trajectory.

State packing: free index f = (l-1)*16 + h*8 + b for layer l, output-half
h (neuron o = h*128 + partition), batch b.  Layer 4 uses h=0, partitions
0..34 only; its dead lanes compute on zeroed/stale-but-finite data and are
never read.
"""

import numpy as np

import concourse.bacc as bacc
import concourse.mybir as mybir
import concourse.dve_ops as dve_ops
from concourse.dve_ops import DveOp
from concourse.dve_spec import Spec, Src0, Src1, C0, C1, lower, _has_src1
from concourse.dve_uop import DveOpSpec
from concourse.tile import TileContext
from concourse.bass_utils import run_bass_kernel_spmd

N_CORES = 8
B_TOT, T = 64, 1000
B = B_TOT // N_CORES              # 8 batches per core
DIMS = [20, 256, 256, 256, 35]
NL = 4
TC = 25                           # timesteps per chunk
NCH = T // TC                     # 40 chunks
SKEW = 2                          # chunk skew between consecutive layers
NW = NCH + SKEW * (NL - 1)        # 46 waves
F = NL * 16                       # packed (layer, half, batch) free width
ALPHA = 0.75
BETA = float(np.float32(1.0 - 0.03))
THETA = 1.25
DT = mybir.dt.float32
AL = mybir.AluOpType


def _lif_ref(in0, in1, s0, s1, imm2):
    return (in0 * (in0 < s0)) * s1 + in1


def _register_lif_op() -> DveOp:
    name = "LIF_STEP_ANT"
    for op in dve_ops.OPS:
        if op.name == name:
            return op
    spec = Spec(body=(Src0 * (Src0 < C0)) * C1 + Src1, reference=_lif_ref)
    row = max(dve_ops._SUB_OPCODE_FOR_NAME.values()) + 1
    assert row < 0x20
    shas = {}
    for ver in ("v3", "v4"):
        s = DveOpSpec(name=name, opcode=row, uops=lower(spec, ver=ver),
                      rd1_en=_has_src1(spec))
        shas[ver] = s.sha(ver)
    op = DveOp(name, spec, subdim=False, uops_sha=shas)
    dve_ops._SUB_OPCODE_FOR_NAME[name] = row
    dve_ops.OPS.append(op)
    dve_ops.CUSTOM_DVE_SPECS[name] = spec
    return op


def _valid_layers(w):
    return [l for l in range(1, NL + 1) if 0 <= w - SKEW * (l - 1) < NCH]


def _flat(ap):
    return ap.rearrange("p a b -> p (a b)")


def _build():
    lif = _register_lif_op()
    nc = bacc.Bacc("TRN2", target_bir_lowering=False, debug=False)

    sp_in = nc.declare_dram_parameter("sp_in", [DIMS[0], NCH, B * TC], DT,
                                      isOutput=False)
    wt_d = [nc.declare_dram_parameter(f"w{l}t", [DIMS[l - 1], DIMS[l]], DT,
                                      isOutput=False)
            for l in range(1, NL + 1)]
    s4_out = nc.declare_dram_parameter("s4_out", [35, NCH, B * TC], DT,
                                       isOutput=True)
    cnt_out = nc.declare_dram_parameter("cnt_out", [128, NL - 1, 16 * TC], DT,
                                        isOutput=True)

    with TileContext(nc) as tc:
        with (
            tc.tile_pool(name="wts", bufs=1) as wts,
            tc.tile_pool(name="state", bufs=1) as state,
            tc.tile_pool(name="psum", bufs=2, space="PSUM") as psum,
        ):
            # persistent weights: w1t [20,256]; w2t/w3t [128,2(Kh),256];
            # w4t [128,2(Kh),35]
            w1t = wts.tile([DIMS[0], 256], DT, name="w1t_s")
            w2t = wts.tile([128, 2, 256], DT, name="w2t_s")
            w3t = wts.tile([128, 2, 256], DT, name="w3t_s")
            w4t = wts.tile([128, 2, 35], DT, name="w4t_s")
            alpha = state.tile([128, F, TC + 1], DT, name="alpha_pat")
            zbuf = [state.tile([128, F, TC + 1], DT, name=f"zbuf{i}")
                    for i in range(3)]
            curb = [state.tile([128, F, TC + 1], DT, name=f"curb{i}")
                    for i in range(3)]
            vpre = [state.tile([128, F, TC], DT, name=f"vpre{i}")
                    for i in range(2)]
            sbft = [state.tile([128, F, TC], DT, name=f"sbft{i}")
                    for i in range(2)]
            spin = [state.tile([DIMS[0], B * TC], DT, name=f"spin{i}")
                    for i in range(3)]
            cnt = state.tile([128, NL - 1, 16 * TC], DT, name="cnt")

            nc.sync.dma_start(out=w1t[:], in_=wt_d[0][:])
            for wtile, d in ((w2t, wt_d[1]), (w3t, wt_d[2]), (w4t, wt_d[3])):
                nc.sync.dma_start(out=wtile[:, 0, :], in_=d[0:128, :])
                nc.sync.dma_start(out=wtile[:, 1, :], in_=d[128:256, :])

            nc.gpsimd.memset(alpha[:], ALPHA)
            nc.gpsimd.memset(alpha[:, :, 0], 0.0)
            for t_ in curb:
                nc.gpsimd.memset(t_[:], 0.0)
            for t_ in zbuf:
                nc.gpsimd.memset(t_[:], 0.0)
            for t_ in vpre:
                nc.vector.memset(t_[:], 0.0)
            nc.vector.memset(cnt[:], 0.0)

            for c in range(min(3, NCH)):
                nc.sync.dma_start(out=spin[c % 3][:], in_=sp_in[:, c, :])

            def emit_post(w):
                """Threshold wave-w vpre into spikes + counts; DMA layer-4."""
                for l in _valid_layers(w):
                    c = w - SKEW * (l - 1)
                    f0 = (l - 1) * 16
                    if l < NL:
                        nc.gpsimd.tensor_scalar(
                            _flat(sbft[w % 2][:, f0:f0 + 16, :]),
                            _flat(vpre[w % 2][:, f0:f0 + 16, :]),
                            THETA, None, AL.is_ge)
                        nc.gpsimd.tensor_tensor(
                            cnt[:, l - 1, :], cnt[:, l - 1, :],
                            _flat(sbft[w % 2][:, f0:f0 + 16, :]), AL.add)
                    else:
                        nc.gpsimd.tensor_scalar(
                            _flat(sbft[w % 2][0:35, f0:f0 + 8, :]),
                            _flat(vpre[w % 2][0:35, f0:f0 + 8, :]),
                            THETA, None, AL.is_ge)
                        nc.sync.dma_start(
                            out=s4_out[:, c, :],
                            in_=_flat(sbft[w % 2][0:35, f0:f0 + 8, :]))

            def emit_prep(w):
                """Matmuls + PSUM copies + carries + scans feeding wave w."""
                for l in _valid_layers(w):
                    c = w - SKEW * (l - 1)
                    f0 = (l - 1) * 16
                    # synapse matmul -> psum
                    if l == 1:
                        zp = psum.tile([128, 2, B * TC], DT, name=f"zp1_{w}",
                                       tag="zp1")
                        for oh in range(2):
                            nc.tensor.matmul(zp[:, oh, :],
                                             w1t[:, oh * 128:(oh + 1) * 128],
                                             spin[c % 3][:],
                                             start=True, stop=True)
                    elif l < NL:
                        zp = psum.tile([128, 2, B * TC], DT, name=f"zp{l}_{w}",
                                       tag=f"zp{l}")
                        wtile = w2t if l == 2 else w3t
                        sp = sbft[(w - SKEW) % 2]
                        fp = (l - 2) * 16
                        for oh in range(2):
                            for kh in range(2):
                                nc.tensor.matmul(
                                    zp[:, oh, :],
                                    wtile[:, kh, oh * 128:(oh + 1) * 128],
                                    _flat(sp[:, fp + kh * 8:fp + kh * 8 + 8, :]),
                                    start=(kh == 0), stop=(kh == 1))
                    else:
                        zp = psum.tile([35, B * TC], DT, name=f"zp4_{w}",
                                       tag="zp4")
                        sp = sbft[(w - SKEW) % 2]
                        fp = (NL - 2) * 16
                        for kh in range(2):
                            nc.tensor.matmul(
                                zp[:],
                                w4t[:, kh, :],
                                _flat(sp[:, fp + kh * 8:fp + kh * 8 + 8, :]),
                                start=(kh == 0), stop=(kh == 1))
                    # PSUM -> zbuf cols 1..TC (ScalarE)
                    zb = zbuf[w % 3]
                    if l < NL:
                        for oh in range(2):
                            nc.scalar.copy(
                                out=zb[:, f0 + oh * 8:f0 + oh * 8 + 8, 1:],
                                in_=zp[:, oh, :].rearrange(
                                    "p (b t) -> p b t", b=B))
                    else:
                        nc.scalar.copy(
                            out=zb[0:35, f0:f0 + 8, 1:],
                            in_=zp[:].rearrange("p (b t) -> p b t", b=B))
                    # carry spacers from previous chunk's cur (ScalarE)
                    nc.scalar.copy(out=zb[:, f0:f0 + 16, 0:1],
                                   in_=curb[(w - 1) % 3][:, f0:f0 + 16,
                                                         TC:TC + 1])
                # one merged segmented current scan over the contiguous
                # valid-layer range (VectorE; Pool lacks the scan opcode)
                vls = _valid_layers(w)
                fa = (vls[0] - 1) * 16
                fb = vls[-1] * 16
                nc.vector.tensor_tensor_scan(
                    _flat(curb[w % 3][:, fa:fb, :]),
                    _flat(alpha[:, 0:fb - fa, :]),
                    _flat(zbuf[w % 3][:, fa:fb, :]),
                    0.0, AL.mult, AL.add)
                # prefetch input chunk w+3 (after the layer-1 matmul has
                # consumed this wave's slot — emission order sets the dep)
                if w + 3 < NCH:
                    nc.sync.dma_start(out=spin[(w + 3) % 3][:],
                                      in_=sp_in[:, w + 3, :])

            def emit_wave(w):
                vls = _valid_layers(w)
                fa, fb = (vls[0] - 1) * 16, vls[-1] * 16
                for j in range(TC):
                    if j > 0:
                        prev = vpre[w % 2][:, fa:fb, j - 1]
                    else:
                        prev = vpre[(w - 1) % 2][:, fa:fb, TC - 1]
                    nc.vector._custom_dve(
                        lif,
                        out=vpre[w % 2][:, fa:fb, j],
                        in0=prev,
                        in1=curb[w % 3][:, fa:fb, j + 1],
                        s0=THETA, s1=BETA)

            for w in range(NW + 2):
                if w >= 2:
                    emit_post(w - 2)
                if w < NW:
                    emit_prep(w)
                    emit_wave(w)

            nc.sync.dma_start(out=cnt_out[:], in_=cnt[:])

    nc.compile()
    return nc


_NC_CACHE = None


def _get_nc():
    global _NC_CACHE
    if _NC_CACHE is None:
        _NC_CACHE = _build()
    return _NC_CACHE


def kernel(spike, W1, W2, W3, W4):
    spike = np.ascontiguousarray(np.asarray(spike, dtype=np.float32))
    wts = {}
    for l, W in enumerate((W1, W2, W3, W4), start=1):
        wts[f"w{l}t"] = np.ascontiguousarray(
            np.asarray(W, dtype=np.float32).T)

    in_maps = []
    for i in range(N_CORES):
        sp = spike[i * B:(i + 1) * B]                      # [B, 20, T]
        sp = sp.reshape(B, DIMS[0], NCH, TC)
        sp = sp.transpose(1, 2, 0, 3).reshape(DIMS[0], NCH, B * TC)
        in_maps.append({"sp_in": np.ascontiguousarray(sp), **wts})

    nc = _get_nc()
    res = None
    for attempt in range(3):
        try:
            res = run_bass_kernel_spmd(nc, in_maps, list(range(N_CORES)))
            break
        except Exception:
            if attempt == 2:
                raise
    assert res is not None

    s4 = np.empty((B_TOT, 35, T), dtype=np.float32)
    cnt_sums = np.zeros(NL, dtype=np.float64)
    for i in range(N_CORES):
        r = res.results[i]
        o = r["s4_out"].reshape(35, NCH, B, TC)            # [35, NCH, B, TC]
        s4[i * B:(i + 1) * B] = (
            o.transpose(2, 0, 1, 3).reshape(B, 35, T))
        cnt_sums[:NL - 1] += r["cnt_out"].astype(np.float64).sum(axis=(0, 2))
    cnt_sums[NL - 1] = s4.sum(dtype=np.float64)

    denom = np.array([B_TOT * DIMS[l] * T for l in range(1, NL + 1)],
                     dtype=np.float64)
    counts = (cnt_sums / denom).astype(np.float32).reshape(1, NL)
    return s4, counts
